# revision 32
# baseline (speedup 1.0000x reference)
"""Trainium2 Bass kernel for nn_CombinedLoss (L1 wave + L1 on real-morlet CWT).

Math: loss = 0.5*mean|o-t| + 0.5*mean|CWT(o)-CWT(t)|.  Convolution is
linear, so CWT(o)-CWT(t) = CWT(d) with d = o-t.

Mapping: width-sharded SPMD (the 36 wavelet widths are distributed over
the 8 cores; every core convolves the full replicated signal with its
4-5 widths).  Each width's banded-Toeplitz conv runs on the tensor
engine as fp8 DoubleRow matmuls: one "unit" contracts 256 consecutive
input samples (2 k-tiles of 128) against a [128,2,128] weight block,
with per-width output shifts S_w = (5w mod 128) chosen so small widths
(1..12) need 1 unit and large widths (13..36) need 2 - 8 units/core.

The moving operand needs k-tile t of output column j to read signal
column (base+t+j); since the PE rejects overlapping-stride APs, the
host supplies the transposed difference signal twice (shift-0/shift-1
planes) so the DoubleRow view is a plain slice.  |.|-sums of the psum
banks are split between DVE (tensor_reduce) and ACT (activation Abs +
accum); per-core partials are combined on the host (the all-reduce).
"""

import numpy as np
import ml_dtypes

import concourse.bass as bass
import concourse.tile as tile
import concourse.mybir as mybir
from concourse.bass_utils import run_bass_kernel_spmd
from concourse.vector_clock import ScopedClock

L = 262144
NW = 36
ALPHA = 0.5
N_CORES = 8
NCOLS = 2048                 # output columns (128 samples each)
PAD = 4                      # zero columns left of the signal
SIGW = 2060                  # PAD + 2048 + 8
WAVE0 = SIGW                 # wave-slice columns start
D2W = SIGW + 256             # + per-core wave slice
F32 = mybir.dt.float32
BF16 = mybir.dt.bfloat16
FP8 = mybir.dt.float8e4
FP8_NP = ml_dtypes.float8_e4m3

# slot structure (identical on every core): (units, c0)
SLOT_UNITS = [2, 2, 2, 1, 1]
SLOT_C0 = [-1, -1, -1, 0, 0]
UNIT_OFF = [0, 2, 4, 6, 7]   # first unit index of each slot
N_UNITS = 8
N_PHASES = 4                 # 4 x 512 output columns
WARMUP_MM = 58

# input DMA blocks (d2 plane-pair column ranges)
D2_BLOCKS = [(0, 520), (520, 1554), (1554, D2W)]

# per-phase psum bank map (slot -> bank) and reduce plan (bank_lo, nbanks,
# engine).  All reduce groups are address-contiguous; triples alternate
# between ACT and DVE so neither engine's chain gates two phases in a row.
BANK_MAP = [
    [0, 1, 2, 3, 4],
    [5, 6, 7, 0, 1],
    [2, 3, 4, 0, 1],
    [5, 6, 7, 0, 1],
]
REDUCE_PLAN = [
    [(0, 1, "dve"), (1, 1, "dve"), (2, 3, "act")],
    [(5, 3, "act"), (0, 1, "dve"), (1, 1, "dve")],
    [(2, 3, "act"), (0, 1, "dve"), (1, 1, "dve")],
    [(5, 3, "act"), (0, 1, "dve"), (1, 1, "dve")],
]


def core_widths(c):
    """5 width slots for core c (0 = zero/padding slot)."""
    return [13 + c, 21 + c, 29 + c, 1 + c, 9 + c if c < 4 else 0]


def _morlet(N, w):
    x = np.linspace(-2.0 * np.pi, 2.0 * np.pi, N)
    return (np.cos(w * x) - np.exp(-0.5 * w * w)) * np.exp(-0.5 * x * x) * np.pi ** (-0.25)


def _build_core_weights(c):
    """[128, 8*256] fp8 weight layout for core c."""
    W = np.zeros((128, N_UNITS, 2, 128), np.float32)
    k = np.arange(128)[:, None]
    i = np.arange(128)[None, :]
    for s, w in enumerate(core_widths(c)):
        if w == 0:
            continue
        N, a0 = 10 * w, 5 * w
        S = a0 % 128
        g = _morlet(N, float(w))
        for u in range(SLOT_UNITS[s]):
            for t in range(2):
                m = k - i - S + a0 + 128 * (SLOT_C0[s] + 2 * u + t)
                W[:, UNIT_OFF[s] + u, t, :] = np.where(
                    (m >= 0) & (m < N), g[np.clip(m, 0, N - 1)], 0.0
                )
    return W.reshape(128, N_UNITS * 256).astype(FP8_NP)


_CORE_WEIGHTS = [_build_core_weights(c) for c in range(N_CORES)]


class _TC(tile.TileContext):
    """TileContext whose tail drain carries at most one sync wait (the
    walrus build in this container rejects multi-wait Drains)."""

    def _lower_ordered_insts(self, ordered):
        nc = self.nc
        for bb_name in list(ordered.keys()):
            insts = ordered[bb_name]
            new = []
            for inst in insts:
                si = inst.sync_info
                if si is not None and len(si.on_wait) > 1:
                    waits = list(si.on_wait)
                    for w in waits[:-1]:
                        nop = mybir.InstEventSemaphore(
                            name=f"wsplit-{nc.next_id()}", ins=[], outs=[],
                            engine=inst.engine,
                        )
                        nop.sync_info = mybir.SyncInfo(on_wait=[w], on_update=[])
                        nc.register_instruction(nop, overwrite=True)
                        new.append(nop)
                    inst.sync_info = mybir.SyncInfo(
                        on_wait=[waits[-1]], on_update=list(si.on_update)
                    )
                new.append(inst)
            ordered[bb_name] = new
        return super()._lower_ordered_insts(ordered)

    def _drain_and_barrier(self, tick_clock, wait_clock):
        nc = self.nc
        probe = mybir.InstDrain(
            name=f"probe-{nc.next_id()}", ins=[], outs=[], engine=mybir.EngineType.SP
        )
        wait_clock.add_sem_waits(probe, ScopedClock({None: tick_clock.global_clock}))
        si = probe.sync_info
        waits = list(si.on_wait) if si is not None else []
        allocated = self.sems.allocated()
        handles = list(allocated.values()) if isinstance(allocated, dict) else list(allocated)
        id2sem = {h.num: h for h in handles}
        name2sem = {h.name: h for h in handles}
        for w in waits:
            sem = id2sem.get(w.id) or name2sem.get(w.ant_name)
            assert sem is not None, (w.id, w.ant_name, sorted(id2sem))
            nc.sync.wait_ge(sem, w.wait_value)
        nc.sync.drain()
        nc.all_engine_barrier()
        popped = nc._tile_sem_poison_stack.pop()
        assert popped is self._sem_poison
        nc.clear_and_free_semaphores(list(self.sems.allocated().values()))
        nc.all_engine_barrier()


_NC_CACHE = None


def _build_nc():
    nc = bass.Bass("TRN2", target_bir_lowering=False, debug=False, num_devices=N_CORES)
    d2_ext = nc.dram_tensor("d2", [128, 2, D2W], FP8, kind="ExternalInput")
    w_ext = nc.dram_tensor("wts", [128, N_UNITS * 256], FP8, kind="ExternalInput")
    out_ext = nc.dram_tensor("partials", [128, 16], F32, kind="ExternalOutput")

    with _TC(nc) as tc:
        with (
            tc.tile_pool(name="sig", bufs=1) as sig_pool,
            tc.tile_pool(name="wt", bufs=1) as wt_pool,
            tc.tile_pool(name="misc", bufs=1) as misc_pool,
            tc.tile_pool(name="ps", bufs=1, space="PSUM") as ps_pool,
        ):
            d2_sb = sig_pool.tile([128, 2, D2W], FP8, tag="d2")
            w_sb = wt_pool.tile([128, N_UNITS, 2, 128], FP8, tag="w")
            scratch = misc_pool.tile([128, 1536], BF16, tag="scr")
            parts = misc_pool.tile([128, 16], F32, tag="parts")
            warm = misc_pool.tile([128, 2, 128], FP8, tag="warm")
            psum = ps_pool.tile([128, 4096], F32, tag="psum")

            # ---- input DMAs (SP engine queue, in priority order) ----
            lo, hi = D2_BLOCKS[0]
            nc.sync.dma_start(d2_sb[:, :, lo:hi], d2_ext[:, :, lo:hi])
            nc.sync.dma_start(w_sb[:, 0:4], w_ext[:, 0:1024])      # slots 0,1
            nc.sync.dma_start(w_sb[:, 4:8], w_ext[:, 1024:2048])   # slots 2,3,4
            for lo, hi in D2_BLOCKS[1:]:
                nc.sync.dma_start(d2_sb[:, :, lo:hi], d2_ext[:, :, lo:hi])

            nc.gpsimd.memset(warm[:], 0.0)
            nc.gpsimd.memset(parts[:], 0.0)

            # ---- PE warmup (p-state ramp bridge; zeros, never read) ----
            for _ in range(WARMUP_MM):
                nc.tensor.matmul(
                    psum[:, 3584:3712], warm[:], warm[:],
                    start=True, stop=True,
                    perf_mode=mybir.MatmulPerfMode.DoubleRow,
                )

            # ---- conv matmuls + reduces, phase by phase ----
            red_col = 1
            for jb in range(N_PHASES):
                for s in range(5):
                    bank = BANK_MAP[jb][s]
                    nu = SLOT_UNITS[s]
                    for u in range(nu):
                        base = PAD + SLOT_C0[s] + 2 * u + 512 * jb
                        nc.tensor.matmul(
                            psum[:, 512 * bank:512 * (bank + 1)],
                            w_sb[:, UNIT_OFF[s] + u],
                            d2_sb[:, :, base:base + 512],
                            start=(u == 0), stop=(u == nu - 1),
                            perf_mode=mybir.MatmulPerfMode.DoubleRow,
                        )
                for lo_bank, nb, eng in REDUCE_PLAN[jb]:
                    src = psum[:, 512 * lo_bank:512 * (lo_bank + nb)]
                    if eng == "dve":
                        nc.vector.tensor_reduce(
                            parts[:, red_col:red_col + 1], src,
                            axis=mybir.AxisListType.X, op=mybir.AluOpType.add,
                            apply_absolute_value=True,
                        )
                    else:
                        nc.scalar.activation(
                            scratch[:, 0:512 * nb], src,
                            mybir.ActivationFunctionType.Abs,
                            accum_out=parts[:, red_col:red_col + 1],
                        )
                    red_col += 1
                if jb == 1:
                    # wave term: |d| over this core's slice, in DVE's idle
                    # window between its ph1 and ph2 pair-reduces
                    nc.vector.tensor_reduce(
                        parts[:, 0:1], d2_sb[:, 0, WAVE0:WAVE0 + 256],
                        axis=mybir.AxisListType.X, op=mybir.AluOpType.add,
                        apply_absolute_value=True,
                    )

            nc.sync.dma_start(out_ext[:], parts[:])
    return nc


def _get_nc():
    global _NC_CACHE
    if _NC_CACHE is None:
        _NC_CACHE = _build_nc()
    return _NC_CACHE


def _make_in_maps(o, t):
    d = (o - t).astype(FP8_NP)
    dT = d.reshape(NCOLS, 128).T                      # [128, 2048]
    d2 = np.zeros((128, 2, D2W), FP8_NP)
    d2[:, 0, PAD:PAD + NCOLS] = dT
    d2[:, 1, PAD - 1:PAD - 1 + NCOLS] = dT            # shift-1 plane

    in_maps = []
    for c in range(N_CORES):
        m = d2.copy()
        m[:, 0, WAVE0:WAVE0 + 256] = dT[:, 256 * c:256 * (c + 1)]
        in_maps.append({"d2": m, "wts": _CORE_WEIGHTS[c]})
    return in_maps


def kernel(outputs, targets):
    o = np.asarray(outputs, dtype=np.float32).reshape(-1)
    t = np.asarray(targets, dtype=np.float32).reshape(-1)
    assert o.shape == (L,) and t.shape == (L,)

    in_maps = _make_in_maps(o, t)
    nc = _get_nc()
    res = run_bass_kernel_spmd(nc, in_maps, core_ids=list(range(N_CORES)))

    wave_sum = 0.0
    cwt_sum = 0.0
    for c in range(N_CORES):
        p = np.asarray(res.results[c]["partials"], dtype=np.float64)
        wave_sum += p[:, 0].sum()
        cwt_sum += p[:, 1:13].sum()
    loss = ALPHA * wave_sum / L + (1.0 - ALPHA) * cwt_sum / (NW * L)
    return np.float32(loss)


# revision 33
# speedup vs baseline: 1.0283x; 1.0283x over previous
"""Trainium2 Bass kernel for nn_CombinedLoss (L1 wave + L1 on real-morlet CWT).

Math: loss = 0.5*mean|o-t| + 0.5*mean|CWT(o)-CWT(t)|.  Convolution is
linear, so CWT(o)-CWT(t) = CWT(d) with d = o-t.

Mapping: width-sharded SPMD (the 36 wavelet widths are distributed over
the 8 cores; every core convolves the full replicated signal with its
4-5 widths).  Each width's banded-Toeplitz conv runs on the tensor
engine as fp8 DoubleRow matmuls: one "unit" contracts 256 consecutive
input samples (2 k-tiles of 128) against a [128,2,128] weight block,
with per-width output shifts S_w = (5w mod 128) chosen so small widths
(1..12) need 1 unit and large widths (13..36) need 2 - 8 units/core.

The moving operand needs k-tile t of output column j to read signal
column (base+t+j); since the PE rejects overlapping-stride APs, the
host supplies the transposed difference signal twice (shift-0/shift-1
planes) so the DoubleRow view is a plain slice.  |.|-sums of the psum
banks are split between DVE (tensor_reduce) and ACT (activation Abs +
accum); per-core partials are combined on the host (the all-reduce).
"""

import numpy as np
import ml_dtypes

import concourse.bass as bass
import concourse.tile as tile
import concourse.mybir as mybir
from concourse.bass_utils import run_bass_kernel_spmd
from concourse.vector_clock import ScopedClock

L = 262144
NW = 36
ALPHA = 0.5
N_CORES = 8
NCOLS = 2048                 # output columns (128 samples each)
PAD = 4                      # zero columns left of the signal
SIGW = 2060                  # PAD + 2048 + 8
WAVE0 = SIGW                 # wave-slice columns start
D2W = SIGW + 256             # + per-core wave slice
F32 = mybir.dt.float32
BF16 = mybir.dt.bfloat16
FP8 = mybir.dt.float8e4
FP8_NP = ml_dtypes.float8_e4m3

# slot structure (identical on every core): (units, c0)
SLOT_UNITS = [2, 2, 2, 1, 1]
SLOT_C0 = [-1, -1, -1, 0, 0]
UNIT_OFF = [0, 2, 4, 6, 7]   # first unit index of each slot
N_UNITS = 8
N_PHASES = 4                 # 4 x 512 output columns
WARMUP_MM = 58

# input DMA blocks (d2 plane-pair column ranges)
D2_BLOCKS = [(0, 520), (520, 1554), (1554, D2W)]

# per-phase psum bank map (slot -> bank) and reduce plan (bank_lo, nbanks,
# engine).  All reduce groups are address-contiguous; triples alternate
# between ACT and DVE so neither engine's chain gates two phases in a row.
BANK_MAP = [
    [0, 1, 2, 3, 4],
    [5, 6, 7, 0, 1],
    [2, 3, 4, 0, 1],
    [5, 6, 7, 0, 1],
]
REDUCE_PLAN = [
    [(0, 2, "dve"), (2, 3, "act")],
    [(5, 3, "act"), (0, 2, "dve")],
    [(2, 3, "act"), (0, 2, "dve")],
    [(5, 3, "act"), (0, 2, "dve")],
]


def core_widths(c):
    """5 width slots for core c (0 = zero/padding slot)."""
    return [13 + c, 21 + c, 29 + c, 1 + c, 9 + c if c < 4 else 0]


def _morlet(N, w):
    x = np.linspace(-2.0 * np.pi, 2.0 * np.pi, N)
    return (np.cos(w * x) - np.exp(-0.5 * w * w)) * np.exp(-0.5 * x * x) * np.pi ** (-0.25)


def _build_core_weights(c):
    """[128, 8*256] fp8 weight layout for core c."""
    W = np.zeros((128, N_UNITS, 2, 128), np.float32)
    k = np.arange(128)[:, None]
    i = np.arange(128)[None, :]
    for s, w in enumerate(core_widths(c)):
        if w == 0:
            continue
        N, a0 = 10 * w, 5 * w
        S = a0 % 128
        g = _morlet(N, float(w))
        for u in range(SLOT_UNITS[s]):
            for t in range(2):
                m = k - i - S + a0 + 128 * (SLOT_C0[s] + 2 * u + t)
                W[:, UNIT_OFF[s] + u, t, :] = np.where(
                    (m >= 0) & (m < N), g[np.clip(m, 0, N - 1)], 0.0
                )
    return W.reshape(128, N_UNITS * 256).astype(FP8_NP)


_CORE_WEIGHTS = [_build_core_weights(c) for c in range(N_CORES)]


class _TC(tile.TileContext):
    """TileContext whose tail drain carries at most one sync wait (the
    walrus build in this container rejects multi-wait Drains)."""

    def _lower_ordered_insts(self, ordered):
        nc = self.nc
        for bb_name in list(ordered.keys()):
            insts = ordered[bb_name]
            new = []
            for inst in insts:
                si = inst.sync_info
                if si is not None and len(si.on_wait) > 1:
                    waits = list(si.on_wait)
                    for w in waits[:-1]:
                        nop = mybir.InstEventSemaphore(
                            name=f"wsplit-{nc.next_id()}", ins=[], outs=[],
                            engine=inst.engine,
                        )
                        nop.sync_info = mybir.SyncInfo(on_wait=[w], on_update=[])
                        nc.register_instruction(nop, overwrite=True)
                        new.append(nop)
                    inst.sync_info = mybir.SyncInfo(
                        on_wait=[waits[-1]], on_update=list(si.on_update)
                    )
                new.append(inst)
            ordered[bb_name] = new
        return super()._lower_ordered_insts(ordered)

    def _drain_and_barrier(self, tick_clock, wait_clock):
        nc = self.nc
        probe = mybir.InstDrain(
            name=f"probe-{nc.next_id()}", ins=[], outs=[], engine=mybir.EngineType.SP
        )
        wait_clock.add_sem_waits(probe, ScopedClock({None: tick_clock.global_clock}))
        si = probe.sync_info
        waits = list(si.on_wait) if si is not None else []
        allocated = self.sems.allocated()
        handles = list(allocated.values()) if isinstance(allocated, dict) else list(allocated)
        id2sem = {h.num: h for h in handles}
        name2sem = {h.name: h for h in handles}
        for w in waits:
            sem = id2sem.get(w.id) or name2sem.get(w.ant_name)
            assert sem is not None, (w.id, w.ant_name, sorted(id2sem))
            nc.sync.wait_ge(sem, w.wait_value)
        nc.sync.drain()
        nc.all_engine_barrier()
        popped = nc._tile_sem_poison_stack.pop()
        assert popped is self._sem_poison
        nc.clear_and_free_semaphores(list(self.sems.allocated().values()))
        nc.all_engine_barrier()


_NC_CACHE = None


def _build_nc():
    nc = bass.Bass("TRN2", target_bir_lowering=False, debug=False, num_devices=N_CORES)
    d2_ext = nc.dram_tensor("d2", [128, 2, D2W], FP8, kind="ExternalInput")
    w_ext = nc.dram_tensor("wts", [128, N_UNITS * 256], FP8, kind="ExternalInput")
    out_ext = nc.dram_tensor("partials", [128, 16], F32, kind="ExternalOutput")

    with _TC(nc) as tc:
        with (
            tc.tile_pool(name="sig", bufs=1) as sig_pool,
            tc.tile_pool(name="wt", bufs=1) as wt_pool,
            tc.tile_pool(name="misc", bufs=1) as misc_pool,
            tc.tile_pool(name="ps", bufs=1, space="PSUM") as ps_pool,
        ):
            d2_sb = sig_pool.tile([128, 2, D2W], FP8, tag="d2")
            w_sb = wt_pool.tile([128, N_UNITS, 2, 128], FP8, tag="w")
            scratch = misc_pool.tile([128, 1536], BF16, tag="scr")
            parts = misc_pool.tile([128, 16], F32, tag="parts")
            warm = misc_pool.tile([128, 2, 128], FP8, tag="warm")
            psum = ps_pool.tile([128, 4096], F32, tag="psum")

            # ---- input DMAs (SP engine queue, in priority order) ----
            lo, hi = D2_BLOCKS[0]
            nc.sync.dma_start(d2_sb[:, :, lo:hi], d2_ext[:, :, lo:hi])
            nc.sync.dma_start(w_sb[:, 0:4], w_ext[:, 0:1024])      # slots 0,1
            nc.sync.dma_start(w_sb[:, 4:8], w_ext[:, 1024:2048])   # slots 2,3,4
            for lo, hi in D2_BLOCKS[1:]:
                nc.sync.dma_start(d2_sb[:, :, lo:hi], d2_ext[:, :, lo:hi])

            nc.gpsimd.memset(warm[:], 0.0)
            nc.gpsimd.memset(parts[:], 0.0)

            # ---- PE warmup (p-state ramp bridge; zeros, never read) ----
            for _ in range(WARMUP_MM):
                nc.tensor.matmul(
                    psum[:, 3584:3712], warm[:], warm[:],
                    start=True, stop=True,
                    perf_mode=mybir.MatmulPerfMode.DoubleRow,
                )

            # ---- conv matmuls + reduces, phase by phase ----
            red_col = 1
            for jb in range(N_PHASES):
                for s in range(5):
                    bank = BANK_MAP[jb][s]
                    nu = SLOT_UNITS[s]
                    for u in range(nu):
                        base = PAD + SLOT_C0[s] + 2 * u + 512 * jb
                        nc.tensor.matmul(
                            psum[:, 512 * bank:512 * (bank + 1)],
                            w_sb[:, UNIT_OFF[s] + u],
                            d2_sb[:, :, base:base + 512],
                            start=(u == 0), stop=(u == nu - 1),
                            perf_mode=mybir.MatmulPerfMode.DoubleRow,
                        )
                for lo_bank, nb, eng in REDUCE_PLAN[jb]:
                    src = psum[:, 512 * lo_bank:512 * (lo_bank + nb)]
                    if eng == "dve":
                        nc.vector.tensor_reduce(
                            parts[:, red_col:red_col + 1], src,
                            axis=mybir.AxisListType.X, op=mybir.AluOpType.add,
                            apply_absolute_value=True,
                        )
                    else:
                        nc.scalar.activation(
                            scratch[:, 0:512 * nb], src,
                            mybir.ActivationFunctionType.Abs,
                            accum_out=parts[:, red_col:red_col + 1],
                        )
                    red_col += 1
                if jb == 1:
                    # wave term: |d| over this core's slice, in DVE's idle
                    # window between its ph1 and ph2 pair-reduces
                    nc.vector.tensor_reduce(
                        parts[:, 0:1], d2_sb[:, 0, WAVE0:WAVE0 + 256],
                        axis=mybir.AxisListType.X, op=mybir.AluOpType.add,
                        apply_absolute_value=True,
                    )

            nc.sync.dma_start(out_ext[:], parts[:])
    return nc


def _get_nc():
    global _NC_CACHE
    if _NC_CACHE is None:
        _NC_CACHE = _build_nc()
    return _NC_CACHE


def _make_in_maps(o, t):
    d = (o - t).astype(FP8_NP)
    dT = d.reshape(NCOLS, 128).T                      # [128, 2048]
    d2 = np.zeros((128, 2, D2W), FP8_NP)
    d2[:, 0, PAD:PAD + NCOLS] = dT
    d2[:, 1, PAD - 1:PAD - 1 + NCOLS] = dT            # shift-1 plane

    in_maps = []
    for c in range(N_CORES):
        m = d2.copy()
        m[:, 0, WAVE0:WAVE0 + 256] = dT[:, 256 * c:256 * (c + 1)]
        in_maps.append({"d2": m, "wts": _CORE_WEIGHTS[c]})
    return in_maps


def kernel(outputs, targets):
    o = np.asarray(outputs, dtype=np.float32).reshape(-1)
    t = np.asarray(targets, dtype=np.float32).reshape(-1)
    assert o.shape == (L,) and t.shape == (L,)

    in_maps = _make_in_maps(o, t)
    nc = _get_nc()
    res = run_bass_kernel_spmd(nc, in_maps, core_ids=list(range(N_CORES)))

    wave_sum = 0.0
    cwt_sum = 0.0
    for c in range(N_CORES):
        p = np.asarray(res.results[c]["partials"], dtype=np.float64)
        wave_sum += p[:, 0].sum()
        cwt_sum += p[:, 1:13].sum()
    loss = ALPHA * wave_sum / L + (1.0 - ALPHA) * cwt_sum / (NW * L)
    return np.float32(loss)
